# revision 19
# baseline (speedup 1.0000x reference)
"""ClusterNorm1d TRN2 kernel (v3: stacked stats, unscaled-cov Newton,
software-pipelined groups).

Math (per cluster k): mu = mean_b x[b,:,k]; cov = centered second moment;
L = chol(cov + eps I); Z = L^-1 (x - mu).

Per core: 32 clusters = 16 pairs, 4 groups of 4 pairs, K-sharded over 8
cores.  Emission is software-pipelined: stats pairs of group g+1 are
interleaved between the Newton steps of group g, so the PE never drains
and the out-DMA stream stays continuous.

  stats: per cluster 16 fp8e4m3 DoubleRow matmuls (K=256) + one K=64
    eps matmul accumulate U^T U into a PAIR-STACKED [128,65] PSUM tile
    (odd cluster written at base partition 64 via M=64 + N=65, which
    keeps the s column but drops the redundant s row).  One ACT copy
    parks the pair slab in st2_all; no re-stacking copies needed.
  Newton (NIT=2) on the UNSCALED Ahat = (B-1) cov = S - s s^T/B
    + eps(B-1)I, with the 1/(B-1) scaling folded into
    scalar_tensor_tensor constants:
      u  = cm o Ahat;  W1^T = 1.5I - u/(B-1);  W1c = 1.5I/(B-1) -
      u^T/(B-1)^2  (= cov-scaled W1, so the last matmul lands at O(1))
      h = Ahat W1^T;  P^ = W1 h;  u2 = cm o P^;
      W2^T = 1.5 W1^T - W1c^T u2      (the true inverse Cholesky^T)
    s s^T comes from the s column via a tiny transpose-matmul
    (s^T = s_col^T @ I) into the quadrant rows, then a K=1 outer
    product.  Quadrant matmuls run concurrently on the PE.
  solve: Z = W x - (W mu) 1^T, bf16 N=512 quadrant matmuls into
    [128,1024] PSUM, bias folded into the PSUM->SBUF copies
    (alternating ACT/DVE); out-DMA on gpsimd (SWDGE), slab-in on sync,
    stats-in on scalar -- three independent queues.
HBM traffic: xs bf16 16.8MB + xq fp8 10.5MB in, 33.5MB f32 out.
Validated vs numpy model: rel_err 4.489e-3 (gate 2e-2).
"""
import sys
sys.path.insert(0, "/opt/trn_rl_repo")

import numpy as np
import ml_dtypes

import concourse.bass as bass
from concourse import bacc
import concourse.mybir as mybir
import concourse.tile as tile
from concourse.bass_utils import run_bass_kernel_spmd

B, D, K, NCORES = 4096, 64, 256, 8
KL = K // NCORES          # clusters per core
NP = KL // 2              # pairs per core
GRP = 2                   # pairs per group
NG = NP // GRP
EPS = 1e-4
NB2 = B // 1024           # solve chunk-pairs per pair
NJ8 = B // 256            # fp8 DoubleRow stats chunks per cluster
SUBW = 80                 # fp8 stats subtile row bytes (65 used, %16)
AF = mybir.ActivationFunctionType
OP = mybir.AluOpType
DR = mybir.MatmulPerfMode.DoubleRow

_cache = {}


def _build_nc():
    nc = bacc.Bacc("TRN2", target_bir_lowering=False, debug=False,
                   num_devices=NCORES)
    d_xs = nc.dram_tensor("xs", [KL, D, B], mybir.dt.bfloat16,
                          kind="ExternalInput")
    d_xq = nc.dram_tensor("xq", [NP, 128, 2 * NJ8 * 2, SUBW],
                          mybir.dt.float8e4, kind="ExternalInput")
    d_cs = nc.dram_tensor("cs", [2 * D, 4 * D], mybir.dt.float32,
                          kind="ExternalInput")
    d_eb = nc.dram_tensor("eb", [D, SUBW], mybir.dt.float8e4,
                          kind="ExternalInput")
    d_out = nc.dram_tensor("out", [KL, D, B], mybir.dt.float32,
                           kind="ExternalOutput")

    inv_b = 1.0 / B
    c2 = 1.0 / (B - 1)
    c4 = c2 * c2
    PW = D + 1            # st2_all width per pair

    with tile.TileContext(nc) as tc:
        with tc.tile_pool(name="consts", bufs=1) as consts, \
             tc.tile_pool(name="slabp", bufs=8) as slabp, \
             tc.tile_pool(name="upool", bufs=4) as upool, \
             tc.tile_pool(name="zpool", bufs=3) as zpool, \
             tc.tile_pool(name="step", bufs=3 * GRP) as step, \
             tc.tile_pool(name="small", bufs=4) as small, \
             tc.tile_pool(name="ps_big", bufs=3, space="PSUM") as ps_big, \
             tc.tile_pool(name="ps_small", bufs=2, space="PSUM") as ps_small:

            tcs = consts.tile([2 * D, 4 * D], mybir.dt.float32)
            nc.sync.dma_start(out=tcs, in_=d_cs.ap())
            id2 = tcs[:, 0:D]
            cm2 = tcs[:, D:2 * D]          # triu(1,k=1) + 0.5 I, stacked
            i15 = tcs[:, 2 * D:3 * D]      # 1.5 I, stacked
            i15c = tcs[:, 3 * D:4 * D]     # 1.5/(B-1) I, stacked
            teb = consts.tile([D, SUBW], mybir.dt.float8e4)
            nc.sync.dma_start(out=teb, in_=d_eb.ap())

            # st2_all[:, 65p : 65p+65]: pair-stacked [S2 | s2]; +pad col
            st2_all = consts.tile([2 * D, PW * NP + 1], mybir.dt.float32)

            # ---- PE warm-up: dense filler matmuls promote the HAM
            # clock gate to 8/8 before the first stats arrive ----
            pwu = ps_small.tile([2 * D, D], mybir.dt.float32, tag="ps64")
            for _ in range(48):
                nc.tensor.matmul(pwu[0:D, :], id2, id2, start=True,
                                 stop=True)

            # ---- prefetch solve slabs (sync/SP ring, FIFO) ----
            slabs = []
            for p in range(NP):
                sl = slabp.tile([2 * D, B], mybir.dt.bfloat16)
                nc.sync.dma_start(
                    out=sl,
                    in_=d_xs.ap()[2 * p:2 * p + 2].rearrange(
                        "c d b -> (c d) b"))
                slabs.append(sl)

            state = {}    # per-pair newton tiles

            def emit_stats_pair(p):
                ub = upool.tile([128, 2 * NJ8 * 2, SUBW],
                                mybir.dt.float8e4)
                nc.scalar.dma_start(out=ub, in_=d_xq.ap()[p])
                for half in range(2):
                    sb = half * NJ8 * 2
                    ps = ps_big.tile([D, PW], mybir.dt.float32, tag="psb")
                    for j in range(NJ8):
                        nc.tensor.matmul(
                            ps, ub[:, sb + 2 * j:sb + 2 * j + 2, 0:D],
                            ub[:, sb + 2 * j:sb + 2 * j + 2, 0:PW],
                            start=(j == 0), stop=False, perf_mode=DR)
                    nc.tensor.matmul(ps, teb[:, 0:D], teb[:, 0:PW],
                                     start=False, stop=True)
                    nc.scalar.copy(
                        st2_all[half * D:(half + 1) * D,
                                PW * p:PW * (p + 1)], ps)

            def emit_newton_a(pairs):
                # s row extract + outer product + amat_hat
                for p in pairs:
                    scol = st2_all[:, PW * p + D:PW * p + D + 1]
                    psr = ps_small.tile([2 * D, D], mybir.dt.float32,
                                        tag="ps64")
                    nc.tensor.matmul(psr[0:1, :], scol[0:D, :], id2[0:D, :],
                                     start=True, stop=True)
                    nc.tensor.matmul(psr[D:D + 1, :], scol[D:2 * D, :],
                                     id2[D:2 * D, :], start=True, stop=True)
                    z2 = step.tile([2 * D, D], mybir.dt.float32, tag="z2")
                    nc.vector.tensor_copy(z2[0:1, :], psr[0:1, :])
                    nc.vector.tensor_copy(z2[D:D + 1, :], psr[D:D + 1, :])
                    state[p] = {"z2": z2}
                for p in pairs:
                    z2 = state[p]["z2"]
                    pso = ps_small.tile([2 * D, D], mybir.dt.float32,
                                        tag="ps64")
                    nc.tensor.matmul(pso[0:D, :], z2[0:1, :], z2[0:1, :],
                                     start=True, stop=True)
                    nc.tensor.matmul(pso[D:2 * D, :], z2[D:D + 1, :],
                                     z2[D:D + 1, :], start=True, stop=True)
                    am = step.tile([2 * D, D], mybir.dt.float32, tag="amat")
                    nc.vector.scalar_tensor_tensor(
                        out=am, in0=pso, scalar=-inv_b,
                        in1=st2_all[:, PW * p:PW * p + D],
                        op0=OP.mult, op1=OP.add)
                    state[p]["am"] = am

            def emit_newton_b(pairs):
                # u = cm o Ahat; wt1 = 1.5I - c2 u; w1c = 1.5c2 I - c4 u^T
                for p in pairs:
                    am = state[p]["am"]
                    u1 = step.tile([2 * D, D], mybir.dt.float32, tag="u1")
                    nc.vector.tensor_mul(u1, cm2, am)
                    psd = ps_small.tile([2 * D, D], mybir.dt.float32,
                                        tag="ps64")
                    nc.tensor.matmul(psd[0:D, :], u1[0:D, :], id2[0:D, :],
                                     start=True, stop=True)
                    nc.tensor.matmul(psd[D:2 * D, :], u1[D:2 * D, :],
                                     id2[D:2 * D, :], start=True, stop=True)
                    w1c = step.tile([2 * D, D], mybir.dt.float32, tag="w1c")
                    nc.vector.scalar_tensor_tensor(
                        out=w1c, in0=psd, scalar=-c4, in1=i15c,
                        op0=OP.mult, op1=OP.add)
                    wt1 = step.tile([2 * D, D], mybir.dt.float32, tag="wt1")
                    nc.vector.scalar_tensor_tensor(
                        out=wt1, in0=u1, scalar=-c2, in1=i15,
                        op0=OP.mult, op1=OP.add)
                    state[p]["w1c"] = w1c
                    state[p]["wt1"] = wt1

            def emit_newton_c1(pairs):
                # h = Ahat wt1
                for p in pairs:
                    am, wt1 = state[p]["am"], state[p]["wt1"]
                    psh = ps_small.tile([2 * D, D], mybir.dt.float32,
                                        tag="ps64")
                    nc.tensor.matmul(psh[0:D, :], am[0:D, :], wt1[0:D, :],
                                     start=True, stop=True)
                    nc.tensor.matmul(psh[D:2 * D, :], am[D:2 * D, :],
                                     wt1[D:2 * D, :], start=True, stop=True)
                    h2 = step.tile([2 * D, D], mybir.dt.float32, tag="h2")
                    nc.scalar.copy(h2, psh)
                    state[p]["h2"] = h2

            def emit_newton_c2(pairs):
                # P^ = W1 h; u2 = cm o P^; wt = 1.5 wt1 - w1c^T u2; cast
                for p in pairs:
                    st = state[p]
                    wt1, h2, w1c = st["wt1"], st["h2"], st["w1c"]
                    psp = ps_small.tile([2 * D, D], mybir.dt.float32,
                                        tag="ps64")
                    nc.tensor.matmul(psp[0:D, :], wt1[0:D, :], h2[0:D, :],
                                     start=True, stop=True)
                    nc.tensor.matmul(psp[D:2 * D, :], wt1[D:2 * D, :],
                                     h2[D:2 * D, :], start=True, stop=True)
                    u2 = step.tile([2 * D, D], mybir.dt.float32, tag="u2")
                    nc.vector.tensor_mul(u2, cm2, psp)
                    pst = ps_small.tile([2 * D, D], mybir.dt.float32,
                                        tag="ps64")
                    nc.tensor.matmul(pst[0:D, :], w1c[0:D, :], u2[0:D, :],
                                     start=True, stop=True)
                    nc.tensor.matmul(pst[D:2 * D, :], w1c[D:2 * D, :],
                                     u2[D:2 * D, :], start=True, stop=True)
                    wt = step.tile([2 * D, D], mybir.dt.float32, tag="wt")
                    nc.vector.scalar_tensor_tensor(
                        out=wt, in0=wt1, scalar=1.5, in1=pst,
                        op0=OP.mult, op1=OP.subtract)
                    wtb = step.tile([2 * D, D], mybir.dt.bfloat16,
                                    tag="wtb")
                    nc.scalar.activation(out=wtb, in_=wt, func=AF.Identity)
                    state[p]["wtb"] = wtb

            def emit_solve_pair(p):
                k0 = 2 * p
                wtb, slab = state[p]["wtb"], slabs[p]
                mub = small.tile([2 * D, 2], mybir.dt.bfloat16, tag="mub")
                nc.scalar.activation(
                    out=mub, in_=st2_all[:, PW * p + D:PW * p + D + 2],
                    func=AF.Identity, scale=inv_b)
                psv = ps_small.tile([2 * D, 2], mybir.dt.float32,
                                    tag="ps64")
                nc.tensor.matmul(psv[0:D, :], wtb[0:D, :], mub[0:D, :],
                                 start=True, stop=True)
                nc.tensor.matmul(psv[D:2 * D, :], wtb[D:2 * D, :],
                                 mub[D:2 * D, :], start=True, stop=True)
                biask = small.tile([2 * D, 1], mybir.dt.float32,
                                   tag="biask")
                nc.scalar.activation(out=biask, in_=psv[:, 0:1],
                                     func=AF.Identity, scale=-1.0)

                zs = zpool.tile([2 * D, B], mybir.dt.float32)
                for j in range(NB2):
                    psz = ps_big.tile([2 * D, 1024], mybir.dt.float32,
                                      tag="psb")
                    for c in range(2):
                        lo = 1024 * j + 512 * c
                        nc.tensor.matmul(
                            psz[0:D, 512 * c:512 * (c + 1)],
                            wtb[0:D, :], slab[0:D, lo:lo + 512],
                            start=True, stop=True)
                        nc.tensor.matmul(
                            psz[D:2 * D, 512 * c:512 * (c + 1)],
                            wtb[D:2 * D, :], slab[D:2 * D, lo:lo + 512],
                            start=True, stop=True)
                    dst = zs[:, 1024 * j:1024 * (j + 1)]
                    if j % 2 == 0:
                        nc.scalar.activation(out=dst, in_=psz,
                                             func=AF.Identity, bias=biask)
                    else:
                        nc.vector.tensor_scalar_add(dst, psz, biask)
                    nc.gpsimd.dma_start(
                        out=d_out.ap()[k0:k0 + 2].rearrange(
                            "c d b -> (c d) b")[:, 1024 * j:1024 * (j + 1)],
                        in_=dst)

            # ---- software-pipelined emission ----
            # per group g: stats of g+1 and solves of g-1 are threaded
            # between the newton steps of g so the PE queue never drains
            # and out-DMA triggers spread evenly.
            groups = [list(range(g * GRP, (g + 1) * GRP)) for g in range(NG)]
            for p in groups[0]:
                emit_stats_pair(p)
            emit_newton_a(groups[0])
            emit_newton_b(groups[0])
            emit_newton_c1(groups[0])
            emit_newton_c2(groups[0])
            for p in groups[0]:
                emit_solve_pair(p)
            for p in groups[1]:
                emit_stats_pair(p)
            for g in range(1, NG):
                nxt = groups[g + 1] if g + 1 < NG else []
                prv = groups[g - 1] if g >= 2 else []
                if nxt:
                    emit_stats_pair(nxt[0])
                emit_newton_a(groups[g])
                if prv:
                    emit_solve_pair(prv[0])
                emit_newton_b(groups[g])
                if nxt:
                    emit_stats_pair(nxt[1])
                emit_newton_c1(groups[g])
                if prv:
                    emit_solve_pair(prv[1])
                emit_newton_c2(groups[g])
            for p in groups[NG - 1]:
                emit_solve_pair(p)

    nc.finalize()
    return nc


def _make_consts():
    ident = np.eye(D, dtype=np.float32)
    cmask = np.triu(np.ones((D, D), np.float32), 1) + 0.5 * ident
    blk = np.concatenate([ident, cmask, 1.5 * ident,
                          (1.5 / (B - 1)) * ident], axis=1)  # [D, 4D]
    return np.concatenate([blk, blk], axis=0)                # [2D, 4D]


def _make_eb():
    eb = np.zeros((D, SUBW), dtype=ml_dtypes.float8_e4m3fn)
    eb[:, 0:D] = (np.sqrt(EPS * (B - 1)) *
                  np.eye(D, dtype=np.float32)).astype(
                      ml_dtypes.float8_e4m3fn)
    return eb


def _prep_inputs(x):
    """x: [B, D, K] fp32 -> per-core input dicts."""
    consts = _make_consts()
    eb = _make_eb()
    in_maps = []
    for c in range(NCORES):
        ks = slice(c * KL, (c + 1) * KL)
        xc = x[:, :, ks]
        xs = np.ascontiguousarray(xc.transpose(2, 1, 0)).astype(
            ml_dtypes.bfloat16)                                 # [KL, D, B]
        xt = xc.transpose(2, 0, 1)                              # [KL, B, D]
        u8 = np.zeros((KL, B, SUBW), dtype=ml_dtypes.float8_e4m3fn)
        u8[:, :, 0:D] = xt.astype(ml_dtypes.float8_e4m3fn)
        u8[:, :, D] = np.float32(1.0)
        # repack to DoubleRow SBUF layout: [NP, 128, 2*NJ8*2, SUBW]
        # sub index = half*2*NJ8 + 2*j + r  <->  b = 256j + 128r + p
        xq = np.ascontiguousarray(
            u8.reshape(NP, 2, NJ8, 2, 128, SUBW)
            .transpose(0, 4, 1, 2, 3, 5)
            .reshape(NP, 128, 2 * NJ8 * 2, SUBW))
        in_maps.append({"xs": xs, "xq": xq, "cs": consts, "eb": eb})
    return in_maps


def _run(x, trace=False):
    if "nc" not in _cache:
        _cache["nc"] = _build_nc()
    nc = _cache["nc"]
    in_maps = _prep_inputs(np.asarray(x, dtype=np.float32))
    res = run_bass_kernel_spmd(nc, in_maps, core_ids=list(range(NCORES)),
                               trace=trace)
    out = np.empty((B, D, K), dtype=np.float32)
    for c in range(NCORES):
        ks = slice(c * KL, (c + 1) * KL)
        out[:, :, ks] = res.results[c]["out"].transpose(2, 1, 0)
    return out, res


def kernel(x):
    out, _ = _run(x, trace=False)
    return out


# revision 20
# speedup vs baseline: 1.0063x; 1.0063x over previous
"""ClusterNorm1d TRN2 kernel (v3: stacked stats, unscaled-cov Newton,
software-pipelined groups).

Math (per cluster k): mu = mean_b x[b,:,k]; cov = centered second moment;
L = chol(cov + eps I); Z = L^-1 (x - mu).

Per core: 32 clusters = 16 pairs, 4 groups of 4 pairs, K-sharded over 8
cores.  Emission is software-pipelined: stats pairs of group g+1 are
interleaved between the Newton steps of group g, so the PE never drains
and the out-DMA stream stays continuous.

  stats: per cluster 16 fp8e4m3 DoubleRow matmuls (K=256) + one K=64
    eps matmul accumulate U^T U into a PAIR-STACKED [128,65] PSUM tile
    (odd cluster written at base partition 64 via M=64 + N=65, which
    keeps the s column but drops the redundant s row).  One ACT copy
    parks the pair slab in st2_all; no re-stacking copies needed.
  Newton (NIT=2) on the UNSCALED Ahat = (B-1) cov = S - s s^T/B
    + eps(B-1)I, with the 1/(B-1) scaling folded into
    scalar_tensor_tensor constants:
      u  = cm o Ahat;  W1^T = 1.5I - u/(B-1);  W1c = 1.5I/(B-1) -
      u^T/(B-1)^2  (= cov-scaled W1, so the last matmul lands at O(1))
      h = Ahat W1^T;  P^ = W1 h;  u2 = cm o P^;
      W2^T = 1.5 W1^T - W1c^T u2      (the true inverse Cholesky^T)
    s s^T comes from the s column via a tiny transpose-matmul
    (s^T = s_col^T @ I) into the quadrant rows, then a K=1 outer
    product.  Quadrant matmuls run concurrently on the PE.
  solve: Z = W x - (W mu) 1^T, bf16 N=512 quadrant matmuls into
    [128,1024] PSUM, bias folded into the PSUM->SBUF copies
    (alternating ACT/DVE); out-DMA on gpsimd (SWDGE), slab-in on sync,
    stats-in on scalar -- three independent queues.
HBM traffic: xs bf16 16.8MB + xq fp8 10.5MB in, 33.5MB f32 out.
Validated vs numpy model: rel_err 4.489e-3 (gate 2e-2).
"""
import sys
sys.path.insert(0, "/opt/trn_rl_repo")

import numpy as np
import ml_dtypes

import concourse.bass as bass
from concourse import bacc
import concourse.mybir as mybir
import concourse.tile as tile
from concourse.bass_utils import run_bass_kernel_spmd

B, D, K, NCORES = 4096, 64, 256, 8
KL = K // NCORES          # clusters per core
NP = KL // 2              # pairs per core
GRP = 2                   # pairs per group
NG = NP // GRP
EPS = 1e-4
NB2 = B // 1024           # solve chunk-pairs per pair
NJ8 = B // 256            # fp8 DoubleRow stats chunks per cluster
SUBW = 80                 # fp8 stats subtile row bytes (65 used, %16)
AF = mybir.ActivationFunctionType
OP = mybir.AluOpType
DR = mybir.MatmulPerfMode.DoubleRow

_cache = {}


def _build_nc():
    nc = bacc.Bacc("TRN2", target_bir_lowering=False, debug=False,
                   num_devices=NCORES)
    d_xs = nc.dram_tensor("xs", [KL, D, B], mybir.dt.bfloat16,
                          kind="ExternalInput")
    d_xq = nc.dram_tensor("xq", [NP, 128, 2 * NJ8 * 2, SUBW],
                          mybir.dt.float8e4, kind="ExternalInput")
    d_cs = nc.dram_tensor("cs", [2 * D, 4 * D], mybir.dt.float32,
                          kind="ExternalInput")
    d_eb = nc.dram_tensor("eb", [D, SUBW], mybir.dt.float8e4,
                          kind="ExternalInput")
    d_out = nc.dram_tensor("out", [KL, D, B], mybir.dt.float32,
                           kind="ExternalOutput")

    inv_b = 1.0 / B
    c2 = 1.0 / (B - 1)
    c4 = c2 * c2
    PW = D + 1            # st2_all width per pair

    with tile.TileContext(nc) as tc:
        with tc.tile_pool(name="consts", bufs=1) as consts, \
             tc.tile_pool(name="slabp", bufs=8) as slabp, \
             tc.tile_pool(name="upool", bufs=4) as upool, \
             tc.tile_pool(name="zpool", bufs=3) as zpool, \
             tc.tile_pool(name="step", bufs=3 * GRP) as step, \
             tc.tile_pool(name="small", bufs=4) as small, \
             tc.tile_pool(name="ps_big", bufs=3, space="PSUM") as ps_big, \
             tc.tile_pool(name="ps_small", bufs=2, space="PSUM") as ps_small:

            tcs = consts.tile([2 * D, 4 * D], mybir.dt.float32)
            nc.sync.dma_start(out=tcs, in_=d_cs.ap())
            id2 = tcs[:, 0:D]
            cm2 = tcs[:, D:2 * D]          # triu(1,k=1) + 0.5 I, stacked
            i15 = tcs[:, 2 * D:3 * D]      # 1.5 I, stacked
            i15c = tcs[:, 3 * D:4 * D]     # 1.5/(B-1) I, stacked
            teb = consts.tile([D, SUBW], mybir.dt.float8e4)
            nc.sync.dma_start(out=teb, in_=d_eb.ap())

            # st2_all[:, 65p : 65p+65]: pair-stacked [S2 | s2]; +pad col
            st2_all = consts.tile([2 * D, PW * NP + 1], mybir.dt.float32)

            # ---- PE warm-up: dense filler matmuls promote the HAM
            # clock gate to 8/8 before the first stats arrive ----
            pwu = ps_small.tile([2 * D, D], mybir.dt.float32, tag="ps64")
            for _ in range(48):
                nc.tensor.matmul(pwu[0:D, :], id2, id2, start=True,
                                 stop=True)

            # ---- prefetch solve slabs (sync/SP ring, FIFO) ----
            slabs = []
            for p in range(NP):
                sl = slabp.tile([2 * D, B], mybir.dt.bfloat16)
                nc.sync.dma_start(
                    out=sl,
                    in_=d_xs.ap()[2 * p:2 * p + 2].rearrange(
                        "c d b -> (c d) b"))
                slabs.append(sl)

            state = {}    # per-pair newton tiles

            def emit_stats_pair(p):
                ub = upool.tile([128, 2 * NJ8 * 2, SUBW],
                                mybir.dt.float8e4)
                nc.scalar.dma_start(out=ub, in_=d_xq.ap()[p])
                for half in range(2):
                    sb = half * NJ8 * 2
                    ps = ps_big.tile([D, PW], mybir.dt.float32, tag="psb")
                    for j in range(NJ8):
                        nc.tensor.matmul(
                            ps, ub[:, sb + 2 * j:sb + 2 * j + 2, 0:D],
                            ub[:, sb + 2 * j:sb + 2 * j + 2, 0:PW],
                            start=(j == 0), stop=False, perf_mode=DR)
                    nc.tensor.matmul(ps, teb[:, 0:D], teb[:, 0:PW],
                                     start=False, stop=True)
                    nc.scalar.copy(
                        st2_all[half * D:(half + 1) * D,
                                PW * p:PW * (p + 1)], ps)

            def emit_newton_a(pairs):
                # s row extract + outer product + amat_hat
                for p in pairs:
                    scol = st2_all[:, PW * p + D:PW * p + D + 1]
                    psr = ps_small.tile([2 * D, D], mybir.dt.float32,
                                        tag="ps64")
                    nc.tensor.matmul(psr[0:1, :], scol[0:D, :], id2[0:D, :],
                                     start=True, stop=True)
                    nc.tensor.matmul(psr[D:D + 1, :], scol[D:2 * D, :],
                                     id2[D:2 * D, :], start=True, stop=True)
                    z2 = step.tile([2 * D, D], mybir.dt.float32, tag="z2")
                    nc.vector.tensor_copy(z2[0:1, :], psr[0:1, :])
                    nc.vector.tensor_copy(z2[D:D + 1, :], psr[D:D + 1, :])
                    state[p] = {"z2": z2}
                for p in pairs:
                    z2 = state[p]["z2"]
                    pso = ps_small.tile([2 * D, D], mybir.dt.float32,
                                        tag="ps64")
                    nc.tensor.matmul(pso[0:D, :], z2[0:1, :], z2[0:1, :],
                                     start=True, stop=True)
                    nc.tensor.matmul(pso[D:2 * D, :], z2[D:D + 1, :],
                                     z2[D:D + 1, :], start=True, stop=True)
                    am = step.tile([2 * D, D], mybir.dt.float32, tag="amat")
                    nc.vector.scalar_tensor_tensor(
                        out=am, in0=pso, scalar=-inv_b,
                        in1=st2_all[:, PW * p:PW * p + D],
                        op0=OP.mult, op1=OP.add)
                    state[p]["am"] = am

            def emit_newton_b(pairs):
                # u = cm o Ahat; wt1 = 1.5I - c2 u; w1c = 1.5c2 I - c4 u^T
                for p in pairs:
                    am = state[p]["am"]
                    u1 = step.tile([2 * D, D], mybir.dt.float32, tag="u1")
                    nc.vector.tensor_mul(u1, cm2, am)
                    psd = ps_small.tile([2 * D, D], mybir.dt.float32,
                                        tag="ps64")
                    nc.tensor.matmul(psd[0:D, :], u1[0:D, :], id2[0:D, :],
                                     start=True, stop=True)
                    nc.tensor.matmul(psd[D:2 * D, :], u1[D:2 * D, :],
                                     id2[D:2 * D, :], start=True, stop=True)
                    w1c = step.tile([2 * D, D], mybir.dt.float32, tag="w1c")
                    nc.vector.scalar_tensor_tensor(
                        out=w1c, in0=psd, scalar=-c4, in1=i15c,
                        op0=OP.mult, op1=OP.add)
                    wt1 = step.tile([2 * D, D], mybir.dt.float32, tag="wt1")
                    nc.vector.scalar_tensor_tensor(
                        out=wt1, in0=u1, scalar=-c2, in1=i15,
                        op0=OP.mult, op1=OP.add)
                    state[p]["w1c"] = w1c
                    state[p]["wt1"] = wt1

            def emit_newton_c1(pairs):
                # h = Ahat wt1
                for p in pairs:
                    am, wt1 = state[p]["am"], state[p]["wt1"]
                    psh = ps_small.tile([2 * D, D], mybir.dt.float32,
                                        tag="ps64")
                    nc.tensor.matmul(psh[0:D, :], am[0:D, :], wt1[0:D, :],
                                     start=True, stop=True)
                    nc.tensor.matmul(psh[D:2 * D, :], am[D:2 * D, :],
                                     wt1[D:2 * D, :], start=True, stop=True)
                    h2 = step.tile([2 * D, D], mybir.dt.float32, tag="h2")
                    nc.scalar.copy(h2, psh)
                    state[p]["h2"] = h2

            def emit_newton_c2(pairs):
                # P^ = W1 h; u2 = cm o P^; wt = 1.5 wt1 - w1c^T u2; cast
                for p in pairs:
                    st = state[p]
                    wt1, h2, w1c = st["wt1"], st["h2"], st["w1c"]
                    psp = ps_small.tile([2 * D, D], mybir.dt.float32,
                                        tag="ps64")
                    nc.tensor.matmul(psp[0:D, :], wt1[0:D, :], h2[0:D, :],
                                     start=True, stop=True)
                    nc.tensor.matmul(psp[D:2 * D, :], wt1[D:2 * D, :],
                                     h2[D:2 * D, :], start=True, stop=True)
                    u2 = step.tile([2 * D, D], mybir.dt.float32, tag="u2")
                    nc.vector.tensor_mul(u2, cm2, psp)
                    pst = ps_small.tile([2 * D, D], mybir.dt.float32,
                                        tag="ps64")
                    nc.tensor.matmul(pst[0:D, :], w1c[0:D, :], u2[0:D, :],
                                     start=True, stop=True)
                    nc.tensor.matmul(pst[D:2 * D, :], w1c[D:2 * D, :],
                                     u2[D:2 * D, :], start=True, stop=True)
                    wt = step.tile([2 * D, D], mybir.dt.float32, tag="wt")
                    nc.vector.scalar_tensor_tensor(
                        out=wt, in0=wt1, scalar=1.5, in1=pst,
                        op0=OP.mult, op1=OP.subtract)
                    wtb = step.tile([2 * D, D], mybir.dt.bfloat16,
                                    tag="wtb")
                    nc.scalar.activation(out=wtb, in_=wt, func=AF.Identity)
                    state[p]["wtb"] = wtb

            def emit_solve_pair(p):
                k0 = 2 * p
                wtb, slab = state[p]["wtb"], slabs[p]
                mub = small.tile([2 * D, 2], mybir.dt.bfloat16, tag="mub")
                nc.scalar.activation(
                    out=mub, in_=st2_all[:, PW * p + D:PW * p + D + 2],
                    func=AF.Identity, scale=inv_b)
                psv = ps_small.tile([2 * D, 2], mybir.dt.float32,
                                    tag="ps64")
                nc.tensor.matmul(psv[0:D, :], wtb[0:D, :], mub[0:D, :],
                                 start=True, stop=True)
                nc.tensor.matmul(psv[D:2 * D, :], wtb[D:2 * D, :],
                                 mub[D:2 * D, :], start=True, stop=True)
                biask = small.tile([2 * D, 1], mybir.dt.float32,
                                   tag="biask")
                nc.scalar.activation(out=biask, in_=psv[:, 0:1],
                                     func=AF.Identity, scale=-1.0)

                zs = zpool.tile([2 * D, B], mybir.dt.float32)
                for j in range(NB2):
                    psz = ps_big.tile([2 * D, 1024], mybir.dt.float32,
                                      tag="psb")
                    for c in range(2):
                        lo = 1024 * j + 512 * c
                        nc.tensor.matmul(
                            psz[0:D, 512 * c:512 * (c + 1)],
                            wtb[0:D, :], slab[0:D, lo:lo + 512],
                            start=True, stop=True)
                        nc.tensor.matmul(
                            psz[D:2 * D, 512 * c:512 * (c + 1)],
                            wtb[D:2 * D, :], slab[D:2 * D, lo:lo + 512],
                            start=True, stop=True)
                    dst = zs[:, 1024 * j:1024 * (j + 1)]
                    if j % 2 == 0:
                        nc.scalar.activation(out=dst, in_=psz,
                                             func=AF.Identity, bias=biask)
                    else:
                        nc.vector.tensor_scalar_add(dst, psz, biask)
                nc.gpsimd.dma_start(
                    out=d_out.ap()[k0:k0 + 2].rearrange("c d b -> (c d) b"),
                    in_=zs)

            # ---- software-pipelined emission ----
            # per group g: stats of g+1 and solves of g-1 are threaded
            # between the newton steps of g so the PE queue never drains
            # and out-DMA triggers spread evenly.
            groups = [list(range(g * GRP, (g + 1) * GRP)) for g in range(NG)]
            for p in groups[0]:
                emit_stats_pair(p)
            emit_newton_a(groups[0])
            emit_newton_b(groups[0])
            emit_newton_c1(groups[0])
            emit_newton_c2(groups[0])
            for p in groups[0]:
                emit_solve_pair(p)
            for p in groups[1]:
                emit_stats_pair(p)
            for g in range(1, NG):
                nxt = groups[g + 1] if g + 1 < NG else []
                prv = groups[g - 1] if g >= 2 else []
                if nxt:
                    emit_stats_pair(nxt[0])
                emit_newton_a(groups[g])
                if prv:
                    emit_solve_pair(prv[0])
                emit_newton_b(groups[g])
                if nxt:
                    emit_stats_pair(nxt[1])
                emit_newton_c1(groups[g])
                if prv:
                    emit_solve_pair(prv[1])
                emit_newton_c2(groups[g])
            for p in groups[NG - 1]:
                emit_solve_pair(p)

    nc.finalize()
    return nc


def _make_consts():
    ident = np.eye(D, dtype=np.float32)
    cmask = np.triu(np.ones((D, D), np.float32), 1) + 0.5 * ident
    blk = np.concatenate([ident, cmask, 1.5 * ident,
                          (1.5 / (B - 1)) * ident], axis=1)  # [D, 4D]
    return np.concatenate([blk, blk], axis=0)                # [2D, 4D]


def _make_eb():
    eb = np.zeros((D, SUBW), dtype=ml_dtypes.float8_e4m3fn)
    eb[:, 0:D] = (np.sqrt(EPS * (B - 1)) *
                  np.eye(D, dtype=np.float32)).astype(
                      ml_dtypes.float8_e4m3fn)
    return eb


def _prep_inputs(x):
    """x: [B, D, K] fp32 -> per-core input dicts."""
    consts = _make_consts()
    eb = _make_eb()
    in_maps = []
    for c in range(NCORES):
        ks = slice(c * KL, (c + 1) * KL)
        xc = x[:, :, ks]
        xs = np.ascontiguousarray(xc.transpose(2, 1, 0)).astype(
            ml_dtypes.bfloat16)                                 # [KL, D, B]
        xt = xc.transpose(2, 0, 1)                              # [KL, B, D]
        u8 = np.zeros((KL, B, SUBW), dtype=ml_dtypes.float8_e4m3fn)
        u8[:, :, 0:D] = xt.astype(ml_dtypes.float8_e4m3fn)
        u8[:, :, D] = np.float32(1.0)
        # repack to DoubleRow SBUF layout: [NP, 128, 2*NJ8*2, SUBW]
        # sub index = half*2*NJ8 + 2*j + r  <->  b = 256j + 128r + p
        xq = np.ascontiguousarray(
            u8.reshape(NP, 2, NJ8, 2, 128, SUBW)
            .transpose(0, 4, 1, 2, 3, 5)
            .reshape(NP, 128, 2 * NJ8 * 2, SUBW))
        in_maps.append({"xs": xs, "xq": xq, "cs": consts, "eb": eb})
    return in_maps


def _run(x, trace=False):
    if "nc" not in _cache:
        _cache["nc"] = _build_nc()
    nc = _cache["nc"]
    in_maps = _prep_inputs(np.asarray(x, dtype=np.float32))
    res = run_bass_kernel_spmd(nc, in_maps, core_ids=list(range(NCORES)),
                               trace=trace)
    out = np.empty((B, D, K), dtype=np.float32)
    for c in range(NCORES):
        ks = slice(c * KL, (c + 1) * KL)
        out[:, :, ks] = res.results[c]["out"].transpose(2, 1, 0)
    return out, res


def kernel(x):
    out, _ = _run(x, trace=False)
    return out


# revision 21
# speedup vs baseline: 1.0929x; 1.0861x over previous
"""ClusterNorm1d TRN2 kernel (v3: stacked stats, unscaled-cov Newton,
software-pipelined groups).

Math (per cluster k): mu = mean_b x[b,:,k]; cov = centered second moment;
L = chol(cov + eps I); Z = L^-1 (x - mu).

Per core: 32 clusters = 16 pairs, 4 groups of 4 pairs, K-sharded over 8
cores.  Emission is software-pipelined: stats pairs of group g+1 are
interleaved between the Newton steps of group g, so the PE never drains
and the out-DMA stream stays continuous.

  stats: per cluster 16 fp8e4m3 DoubleRow matmuls (K=256) + one K=64
    eps matmul accumulate U^T U into a PAIR-STACKED [128,65] PSUM tile
    (odd cluster written at base partition 64 via M=64 + N=65, which
    keeps the s column but drops the redundant s row).  One ACT copy
    parks the pair slab in st2_all; no re-stacking copies needed.
  Newton (NIT=2) on the UNSCALED Ahat = (B-1) cov = S - s s^T/B
    + eps(B-1)I, with the 1/(B-1) scaling folded into
    scalar_tensor_tensor constants:
      u  = cm o Ahat;  W1^T = 1.5I - u/(B-1);  W1c = 1.5I/(B-1) -
      u^T/(B-1)^2  (= cov-scaled W1, so the last matmul lands at O(1))
      h = Ahat W1^T;  P^ = W1 h;  u2 = cm o P^;
      W2^T = 1.5 W1^T - W1c^T u2      (the true inverse Cholesky^T)
    s s^T comes from the s column via a tiny transpose-matmul
    (s^T = s_col^T @ I) into the quadrant rows, then a K=1 outer
    product.  Quadrant matmuls run concurrently on the PE.
  solve: Z = W x - (W mu) 1^T, bf16 N=512 quadrant matmuls into
    [128,1024] PSUM, bias folded into the PSUM->SBUF copies
    (alternating ACT/DVE); out-DMA on gpsimd (SWDGE), slab-in on sync,
    stats-in on scalar -- three independent queues.
HBM traffic: xs bf16 16.8MB + xq fp8 10.5MB in, 33.5MB f32 out.
Validated vs numpy model: rel_err 4.489e-3 (gate 2e-2).
"""
import sys
sys.path.insert(0, "/opt/trn_rl_repo")

import numpy as np
import ml_dtypes

import concourse.bass as bass
from concourse import bacc
import concourse.mybir as mybir
import concourse.tile as tile
from concourse.bass_utils import run_bass_kernel_spmd

B, D, K, NCORES = 4096, 64, 256, 8
KL = K // NCORES          # clusters per core
NP = KL // 2              # pairs per core
GRP = 2                   # pairs per group
NG = NP // GRP
EPS = 1e-4
NB2 = B // 1024           # solve chunk-pairs per pair
NJ8 = B // 256            # fp8 DoubleRow stats chunks per cluster
SUBW = 80                 # fp8 stats subtile row bytes (65 used, %16)
AF = mybir.ActivationFunctionType
OP = mybir.AluOpType
DR = mybir.MatmulPerfMode.DoubleRow

_cache = {}


def _build_nc():
    nc = bacc.Bacc("TRN2", target_bir_lowering=False, debug=False,
                   num_devices=NCORES)
    d_xs = nc.dram_tensor("xs", [KL, D, B], mybir.dt.bfloat16,
                          kind="ExternalInput")
    d_xq = nc.dram_tensor("xq", [NP, 128, 2 * NJ8 * 2, SUBW],
                          mybir.dt.float8e4, kind="ExternalInput")
    d_cs = nc.dram_tensor("cs", [2 * D, 4 * D], mybir.dt.float32,
                          kind="ExternalInput")
    d_eb = nc.dram_tensor("eb", [D, SUBW], mybir.dt.float8e4,
                          kind="ExternalInput")
    d_out = nc.dram_tensor("out", [KL, D, B], mybir.dt.float32,
                           kind="ExternalOutput")

    inv_b = 1.0 / B
    c2 = 1.0 / (B - 1)
    c4 = c2 * c2
    PW = D + 1            # st2_all width per pair

    with tile.TileContext(nc) as tc:
        with tc.tile_pool(name="consts", bufs=1) as consts, \
             tc.tile_pool(name="slabp", bufs=8) as slabp, \
             tc.tile_pool(name="upool", bufs=4) as upool, \
             tc.tile_pool(name="zpool", bufs=3) as zpool, \
             tc.tile_pool(name="step", bufs=3 * GRP) as step, \
             tc.tile_pool(name="small", bufs=4) as small, \
             tc.tile_pool(name="ps_big", bufs=3, space="PSUM") as ps_big, \
             tc.tile_pool(name="ps_small", bufs=2, space="PSUM") as ps_small:

            tcs = consts.tile([2 * D, 4 * D], mybir.dt.float32)
            nc.sync.dma_start(out=tcs, in_=d_cs.ap())
            id2 = tcs[:, 0:D]
            cm2 = tcs[:, D:2 * D]          # triu(1,k=1) + 0.5 I, stacked
            i15 = tcs[:, 2 * D:3 * D]      # 1.5 I, stacked
            i15c = tcs[:, 3 * D:4 * D]     # 1.5/(B-1) I, stacked
            teb = consts.tile([D, SUBW], mybir.dt.float8e4)
            nc.sync.dma_start(out=teb, in_=d_eb.ap())

            # st2_all[:, 65p : 65p+65]: pair-stacked [S2 | s2]; +pad col
            st2_all = consts.tile([2 * D, PW * NP + 1], mybir.dt.float32)

            # ---- prefetch solve slabs (sync/SP ring, FIFO) ----
            slabs = []
            for p in range(NP):
                sl = slabp.tile([2 * D, B], mybir.dt.bfloat16)
                nc.sync.dma_start(
                    out=sl,
                    in_=d_xs.ap()[2 * p:2 * p + 2].rearrange(
                        "c d b -> (c d) b"))
                slabs.append(sl)

            state = {}    # per-pair newton tiles

            def emit_stats_pair(p):
                ub = upool.tile([128, 2 * NJ8 * 2, SUBW],
                                mybir.dt.float8e4)
                nc.scalar.dma_start(out=ub, in_=d_xq.ap()[p])
                for half in range(2):
                    sb = half * NJ8 * 2
                    ps = ps_big.tile([D, PW], mybir.dt.float32, tag="psb")
                    for j in range(NJ8):
                        nc.tensor.matmul(
                            ps, ub[:, sb + 2 * j:sb + 2 * j + 2, 0:D],
                            ub[:, sb + 2 * j:sb + 2 * j + 2, 0:PW],
                            start=(j == 0), stop=False, perf_mode=DR)
                    nc.tensor.matmul(ps, teb[:, 0:D], teb[:, 0:PW],
                                     start=False, stop=True)
                    nc.scalar.copy(
                        st2_all[half * D:(half + 1) * D,
                                PW * p:PW * (p + 1)], ps)

            def emit_newton_a(pairs):
                # s row extract + outer product + amat_hat
                for p in pairs:
                    scol = st2_all[:, PW * p + D:PW * p + D + 1]
                    psr = ps_small.tile([2 * D, D], mybir.dt.float32,
                                        tag="ps64")
                    nc.tensor.matmul(psr[0:1, :], scol[0:D, :], id2[0:D, :],
                                     start=True, stop=True)
                    nc.tensor.matmul(psr[D:D + 1, :], scol[D:2 * D, :],
                                     id2[D:2 * D, :], start=True, stop=True)
                    z2 = step.tile([2 * D, D], mybir.dt.float32, tag="z2")
                    nc.vector.tensor_copy(z2[0:1, :], psr[0:1, :])
                    nc.vector.tensor_copy(z2[D:D + 1, :], psr[D:D + 1, :])
                    state[p] = {"z2": z2}
                for p in pairs:
                    z2 = state[p]["z2"]
                    pso = ps_small.tile([2 * D, D], mybir.dt.float32,
                                        tag="ps64")
                    nc.tensor.matmul(pso[0:D, :], z2[0:1, :], z2[0:1, :],
                                     start=True, stop=True)
                    nc.tensor.matmul(pso[D:2 * D, :], z2[D:D + 1, :],
                                     z2[D:D + 1, :], start=True, stop=True)
                    am = step.tile([2 * D, D], mybir.dt.float32, tag="amat")
                    nc.vector.scalar_tensor_tensor(
                        out=am, in0=pso, scalar=-inv_b,
                        in1=st2_all[:, PW * p:PW * p + D],
                        op0=OP.mult, op1=OP.add)
                    state[p]["am"] = am

            def emit_newton_b(pairs):
                # u = cm o Ahat; wt1 = 1.5I - c2 u; w1c = 1.5c2 I - c4 u^T
                for p in pairs:
                    am = state[p]["am"]
                    u1 = step.tile([2 * D, D], mybir.dt.float32, tag="u1")
                    nc.vector.tensor_mul(u1, cm2, am)
                    psd = ps_small.tile([2 * D, D], mybir.dt.float32,
                                        tag="ps64")
                    nc.tensor.matmul(psd[0:D, :], u1[0:D, :], id2[0:D, :],
                                     start=True, stop=True)
                    nc.tensor.matmul(psd[D:2 * D, :], u1[D:2 * D, :],
                                     id2[D:2 * D, :], start=True, stop=True)
                    w1c = step.tile([2 * D, D], mybir.dt.float32, tag="w1c")
                    nc.vector.scalar_tensor_tensor(
                        out=w1c, in0=psd, scalar=-c4, in1=i15c,
                        op0=OP.mult, op1=OP.add)
                    wt1 = step.tile([2 * D, D], mybir.dt.float32, tag="wt1")
                    nc.vector.scalar_tensor_tensor(
                        out=wt1, in0=u1, scalar=-c2, in1=i15,
                        op0=OP.mult, op1=OP.add)
                    state[p]["w1c"] = w1c
                    state[p]["wt1"] = wt1

            def emit_newton_c1(pairs):
                # h = Ahat wt1
                for p in pairs:
                    am, wt1 = state[p]["am"], state[p]["wt1"]
                    psh = ps_small.tile([2 * D, D], mybir.dt.float32,
                                        tag="ps64")
                    nc.tensor.matmul(psh[0:D, :], am[0:D, :], wt1[0:D, :],
                                     start=True, stop=True)
                    nc.tensor.matmul(psh[D:2 * D, :], am[D:2 * D, :],
                                     wt1[D:2 * D, :], start=True, stop=True)
                    h2 = step.tile([2 * D, D], mybir.dt.float32, tag="h2")
                    nc.scalar.copy(h2, psh)
                    state[p]["h2"] = h2

            def emit_newton_c2(pairs):
                # P^ = W1 h; u2 = cm o P^; wt = 1.5 wt1 - w1c^T u2; cast
                for p in pairs:
                    st = state[p]
                    wt1, h2, w1c = st["wt1"], st["h2"], st["w1c"]
                    psp = ps_small.tile([2 * D, D], mybir.dt.float32,
                                        tag="ps64")
                    nc.tensor.matmul(psp[0:D, :], wt1[0:D, :], h2[0:D, :],
                                     start=True, stop=True)
                    nc.tensor.matmul(psp[D:2 * D, :], wt1[D:2 * D, :],
                                     h2[D:2 * D, :], start=True, stop=True)
                    u2 = step.tile([2 * D, D], mybir.dt.float32, tag="u2")
                    nc.vector.tensor_mul(u2, cm2, psp)
                    pst = ps_small.tile([2 * D, D], mybir.dt.float32,
                                        tag="ps64")
                    nc.tensor.matmul(pst[0:D, :], w1c[0:D, :], u2[0:D, :],
                                     start=True, stop=True)
                    nc.tensor.matmul(pst[D:2 * D, :], w1c[D:2 * D, :],
                                     u2[D:2 * D, :], start=True, stop=True)
                    wt = step.tile([2 * D, D], mybir.dt.float32, tag="wt")
                    nc.vector.scalar_tensor_tensor(
                        out=wt, in0=wt1, scalar=1.5, in1=pst,
                        op0=OP.mult, op1=OP.subtract)
                    wtb = step.tile([2 * D, D], mybir.dt.bfloat16,
                                    tag="wtb")
                    nc.scalar.activation(out=wtb, in_=wt, func=AF.Identity)
                    state[p]["wtb"] = wtb

            def emit_solve_pair(p):
                k0 = 2 * p
                wtb, slab = state[p]["wtb"], slabs[p]
                mub = small.tile([2 * D, 2], mybir.dt.bfloat16, tag="mub")
                nc.scalar.activation(
                    out=mub, in_=st2_all[:, PW * p + D:PW * p + D + 2],
                    func=AF.Identity, scale=inv_b)
                psv = ps_small.tile([2 * D, 2], mybir.dt.float32,
                                    tag="ps64")
                nc.tensor.matmul(psv[0:D, :], wtb[0:D, :], mub[0:D, :],
                                 start=True, stop=True)
                nc.tensor.matmul(psv[D:2 * D, :], wtb[D:2 * D, :],
                                 mub[D:2 * D, :], start=True, stop=True)
                biask = small.tile([2 * D, 1], mybir.dt.float32,
                                   tag="biask")
                nc.scalar.activation(out=biask, in_=psv[:, 0:1],
                                     func=AF.Identity, scale=-1.0)

                zs = zpool.tile([2 * D, B], mybir.dt.float32)
                for j in range(NB2):
                    psz = ps_big.tile([2 * D, 1024], mybir.dt.float32,
                                      tag="psb")
                    for c in range(2):
                        lo = 1024 * j + 512 * c
                        nc.tensor.matmul(
                            psz[0:D, 512 * c:512 * (c + 1)],
                            wtb[0:D, :], slab[0:D, lo:lo + 512],
                            start=True, stop=True)
                        nc.tensor.matmul(
                            psz[D:2 * D, 512 * c:512 * (c + 1)],
                            wtb[D:2 * D, :], slab[D:2 * D, lo:lo + 512],
                            start=True, stop=True)
                    dst = zs[:, 1024 * j:1024 * (j + 1)]
                    if j % 2 == 0:
                        nc.scalar.activation(out=dst, in_=psz,
                                             func=AF.Identity, bias=biask)
                    else:
                        nc.vector.tensor_scalar_add(dst, psz, biask)
                nc.gpsimd.dma_start(
                    out=d_out.ap()[k0:k0 + 2].rearrange("c d b -> (c d) b"),
                    in_=zs)

            # ---- software-pipelined emission ----
            # per group g: stats of g+1 and solves of g-1 are threaded
            # between the newton steps of g so the PE queue never drains
            # and out-DMA triggers spread evenly.
            groups = [list(range(g * GRP, (g + 1) * GRP)) for g in range(NG)]
            for p in groups[0]:
                emit_stats_pair(p)
            emit_newton_a(groups[0])
            emit_newton_b(groups[0])
            emit_newton_c1(groups[0])
            emit_newton_c2(groups[0])
            for p in groups[0]:
                emit_solve_pair(p)
            for p in groups[1]:
                emit_stats_pair(p)
            for g in range(1, NG):
                nxt = groups[g + 1] if g + 1 < NG else []
                prv = groups[g - 1] if g >= 2 else []
                if nxt:
                    emit_stats_pair(nxt[0])
                emit_newton_a(groups[g])
                if prv:
                    emit_solve_pair(prv[0])
                emit_newton_b(groups[g])
                if nxt:
                    emit_stats_pair(nxt[1])
                emit_newton_c1(groups[g])
                if prv:
                    emit_solve_pair(prv[1])
                emit_newton_c2(groups[g])
            for p in groups[NG - 1]:
                emit_solve_pair(p)

    nc.finalize()
    return nc


def _make_consts():
    ident = np.eye(D, dtype=np.float32)
    cmask = np.triu(np.ones((D, D), np.float32), 1) + 0.5 * ident
    blk = np.concatenate([ident, cmask, 1.5 * ident,
                          (1.5 / (B - 1)) * ident], axis=1)  # [D, 4D]
    return np.concatenate([blk, blk], axis=0)                # [2D, 4D]


def _make_eb():
    eb = np.zeros((D, SUBW), dtype=ml_dtypes.float8_e4m3fn)
    eb[:, 0:D] = (np.sqrt(EPS * (B - 1)) *
                  np.eye(D, dtype=np.float32)).astype(
                      ml_dtypes.float8_e4m3fn)
    return eb


def _prep_inputs(x):
    """x: [B, D, K] fp32 -> per-core input dicts."""
    consts = _make_consts()
    eb = _make_eb()
    in_maps = []
    for c in range(NCORES):
        ks = slice(c * KL, (c + 1) * KL)
        xc = x[:, :, ks]
        xs = np.ascontiguousarray(xc.transpose(2, 1, 0)).astype(
            ml_dtypes.bfloat16)                                 # [KL, D, B]
        xt = xc.transpose(2, 0, 1)                              # [KL, B, D]
        u8 = np.zeros((KL, B, SUBW), dtype=ml_dtypes.float8_e4m3fn)
        u8[:, :, 0:D] = xt.astype(ml_dtypes.float8_e4m3fn)
        u8[:, :, D] = np.float32(1.0)
        # repack to DoubleRow SBUF layout: [NP, 128, 2*NJ8*2, SUBW]
        # sub index = half*2*NJ8 + 2*j + r  <->  b = 256j + 128r + p
        xq = np.ascontiguousarray(
            u8.reshape(NP, 2, NJ8, 2, 128, SUBW)
            .transpose(0, 4, 1, 2, 3, 5)
            .reshape(NP, 128, 2 * NJ8 * 2, SUBW))
        in_maps.append({"xs": xs, "xq": xq, "cs": consts, "eb": eb})
    return in_maps


def _run(x, trace=False):
    if "nc" not in _cache:
        _cache["nc"] = _build_nc()
    nc = _cache["nc"]
    in_maps = _prep_inputs(np.asarray(x, dtype=np.float32))
    res = run_bass_kernel_spmd(nc, in_maps, core_ids=list(range(NCORES)),
                               trace=trace)
    out = np.empty((B, D, K), dtype=np.float32)
    for c in range(NCORES):
        ks = slice(c * KL, (c + 1) * KL)
        out[:, :, ks] = res.results[c]["out"].transpose(2, 1, 0)
    return out, res


def kernel(x):
    out, _ = _run(x, trace=False)
    return out
